# revision 53
# baseline (speedup 1.0000x reference)
"""Trainium2 Bass kernel for BasicEuclideanDistModel (gnn_message_passing).

Math:
  result = sum_e (beta - ||dz_e + dv_e t_e||)
           - dt * sum_{i<j, s} exp(beta - ||z_i(t_s) - z_j(t_s)||)

Device strategy (8 cores, one uniform SPMD program; per-core variation
lives entirely in the input DATA):

* Non-event term. The 10-point midpoint Riemann sum over t is replaced
  by a 4-node Chebyshev evaluation: G(t) = sum_pairs exp(-d(t)) is an
  analytic function of t, so sum_s G(t_s) = sum_m w_m G(tau_m) with
  Lagrange weights w (host-side; measured interp error ~5e-6 relative,
  tolerance is 2e-2). 2.5x less pairwise work than the reference grid.

  The upper triangle is cut into 16 strips (i-block b x j >= 128*b);
  core c owns strips c and 15-c, which is EXACTLY 2176 j-columns for
  every core - zero padding. d^2(i,j,t) = F_i(t).G_j as a K=16 fp32r
  inner product, where rows 0:8 carry strip-A's F/G features and rows
  8:16 strip-B's, the inactive half zeroed per column (host-packed).
  One stationary [16,128] per sample covers both strips; 5 matmuls of
  <=512 cols fill PSUM; DVE relu (PSUM f32 -> bf16), then two in-place
  ACT passes (sqrt, exp w/ fused per-partition row sums). Each strip's
  own diagonal 128-block sits at a static column (0:128, 128:256), is
  summed by a tiny DVE reduce, and the host subtracts the half-counted
  duplicates and self-pairs.

* Event term: 25000 events/core packed SoA bf16 [128, 9, 196]
  (zu, vu, zv, vv, t); pure DVE algebra -> d^2, one ACT sqrt with
  fused row-sum. Pad slots have zu=zv, t=0 -> d=0.

* Host marshalling is O(N)+O(E) data prep only (feature polynomials,
  event packing); all O(N^2 * S) and O(E) arithmetic runs on device.
  beta folded in on host: sum exp(beta-d) = e^beta sum exp(-d);
  sum(beta-d) = E beta - sum d.
"""

import os

import numpy as np


def _import_concourse():
    try:
        import concourse  # noqa: F401
    except ImportError:
        import sys

        for p in ("/opt/trn_rl_repo", "/root/.axon_site/_ro/trn_rl_repo"):
            if os.path.isdir(p) and p not in sys.path:
                sys.path.insert(0, p)


_import_concourse()

from contextlib import ExitStack  # noqa: E402

import concourse.bacc as bacc  # noqa: E402
import concourse.mybir as mybir  # noqa: E402
import concourse.tile as tile  # noqa: E402
from concourse.tile_rust import add_dep_helper  # noqa: E402

N = 2048          # nodes
NB = 16           # 128-row blocks
SREF = 10         # reference Riemann samples
M = 2             # Gauss sample nodes actually evaluated
NCORES = 8
JC = 2176         # j-columns per core: (2048-128t) + 128(t+1), exact
EV_PER_CORE = 200000 // NCORES       # 25000 real events per core
C_EV = 196        # event slots per partition (128*196 = 25088 >= 25000)
FEATW = JC + M * 128                 # combined feature input width

F32 = mybir.dt.float32
F32R = mybir.dt.float32r
BF16 = mybir.dt.bfloat16
AF = mybir.ActivationFunctionType
OP = mybir.AluOpType

# HW ACT Sqrt(x<0) = NaN (measured), and fp32r rounding pushes near-zero
# d^2 as low as -5.7e-4 (measured) - so d^2 MUST be relu'd before sqrt.
# A sqrt(x + delta) bias instead of relu costs delta/2*sum(exp(-d)/d)
# ~ 1.3e-2 relative at delta=1e-2 (measured) - too close to tolerance.
DELTA = 0.0
DEBUG_MIN = os.environ.get("BASSK_DEBUG_MIN") == "1"

_CACHE: dict = {}


def _build():
    if "nc" in _CACHE:
        return _CACHE["nc"]

    nc = bacc.Bacc(
        "TRN2", target_bir_lowering=False, debug=False, enable_asserts=False,
    )

    feat_d = nc.dram_tensor("feat", [16, FEATW], F32R, kind="ExternalInput").ap()
    ev_d = nc.dram_tensor("ev", [128, 5 * C_EV // 2], F32, kind="ExternalInput").ap()
    out_d = nc.dram_tensor("outp", [1, 16], F32, kind="ExternalOutput").ap()
    if DEBUG_MIN:
        accd_d = nc.dram_tensor("accd", [128, 16], F32,
                                kind="ExternalOutput").ap()

    with tile.TileContext(nc) as tc, ExitStack() as ctx:
        cpool = ctx.enter_context(tc.tile_pool(name="const", bufs=1))

        # feature load first (gates PE); events ride the scalar-engine
        # HWDGE queue in parallel
        # feat split across the two HWDGE queues (sync + scalar), with
        # the stationaries + first matmul columns in the sync half so
        # the first matmuls depend on one queue only; the event data
        # goes through gpsimd SWDGE with its OWN completion semaphore -
        # when it shared a queue with feat the legalized PE wait covered
        # the big event DMA and the first matmul stalled ~5us past
        # feat's landing
        feat = cpool.tile([16, FEATW], F32R)
        half = M * 128 + 1024
        nc.sync.dma_start(feat[:, 0:half], feat_d[:, 0:half])
        nc.scalar.dma_start(feat[:, half:FEATW], feat_d[:, half:FEATW])
        evt = cpool.tile([128, 5 * C_EV // 2], F32)
        nc.gpsimd.dma_start(evt[:], ev_d)

        l2 = feat[:, 0:M * 128].rearrange(
            "p (m c) -> p m c", c=128
        )                                                # [16, M, 128] F rows
        t2 = feat[:, M * 128:FEATW]                      # [16, 2176] G cols

        acc = cpool.tile([128, 16], F32)
        nc.vector.memset(acc[:], 0.0)
        ones = cpool.tile([128, 1], F32)
        nc.vector.memset(ones[:], 1.0)

        # ---------------- pairwise: matmul -> relu -> sqrt -> exp ----
        with tc.tile_pool(name="bigq", bufs=3, space="PSUM") as bigq, \
                tc.tile_pool(name="smallq", bufs=1, space="PSUM") as smallq, \
                tc.tile_pool(name="rsq", bufs=1, space="PSUM") as rsq, \
                tc.tile_pool(name="wp", bufs=M) as wpool, \
                tc.tile_pool(name="sp", bufs=1) as spool:
            w_tiles = []
            sq_insts = []
            for m in range(M):
                lm = l2[:, m, :]
                w = wpool.tile([128, JC], BF16, tag="w", name="w")
                qa = bigq.tile([128, 2, 512], F32, tag="q", name="qa")
                nc.tensor.matmul(qa[:, 0, :], lm, t2[:, 0:512],
                                 start=True, stop=True)
                nc.tensor.matmul(qa[:, 1, :], lm, t2[:, 512:1024],
                                 start=True, stop=True)
                qb = bigq.tile([128, 2, 512], F32, tag="q", name="qb")
                nc.tensor.matmul(qb[:, 0, :], lm, t2[:, 1024:1536],
                                 start=True, stop=True)
                nc.tensor.matmul(qb[:, 1, :], lm, t2[:, 1536:2048],
                                 start=True, stop=True)
                qs = smallq.tile([128, 128], F32, tag="qs", name="qs")
                nc.tensor.matmul(qs[:], lm, t2[:, 2048:JC],
                                 start=True, stop=True)
                # relu clamps fp32r-rounding negatives ahead of ACT sqrt.
                # Sample 0 front is fine-chunked so ACT starts ~1us
                # earlier; the qb+qs sqrt merges into one [1152] op
                # (per-op overhead is ~300ns on ACT).
                if m == 0:
                    nc.vector.tensor_scalar_max(
                        w[:, 0:512], qa[:, 0, :], 0.0)
                    sq_insts.append(nc.scalar.activation(
                        w[:, 0:512], w[:, 0:512], AF.Sqrt))
                    nc.vector.tensor_scalar_max(
                        w[:, 512:1024], qa[:, 1, :], 0.0)
                    sq_insts.append(nc.scalar.activation(
                        w[:, 512:1024], w[:, 512:1024], AF.Sqrt))
                else:
                    nc.vector.tensor_scalar_max(w[:, 0:1024], qa[:], 0.0)
                    sq_insts.append(nc.scalar.activation(
                        w[:, 0:1024], w[:, 0:1024], AF.Sqrt))
                nc.vector.tensor_scalar_max(w[:, 1024:2048], qb[:], 0.0)
                nc.vector.tensor_scalar_max(w[:, 2048:JC], qs[:], 0.0)
                sq_insts.append(nc.scalar.activation(
                    w[:, 1024:JC], w[:, 1024:JC], AF.Sqrt))
                if DEBUG_MIN:
                    nc.vector.tensor_reduce(
                        acc[:, 13 + m:14 + m],
                        qa[:].rearrange("p a b -> p (a b)"),
                        axis=mybir.AxisListType.X, op=OP.min)
                w_tiles.append(w)

            # -------- event term (Pool-engine algebra + one ACT sqrt) ----
            # host packs per-event [dzx, dzy, dvx, dvy, t]; the algebra
            # runs on gpsimd (SBUF-only ops) so it's ready as soon as the
            # event DMA lands, in parallel with the DVE relus. d^2 comes
            # from SQUARES, so it is non-negative by construction (ACT
            # Sqrt of a negative is NaN).
            evb = evt[:].bitcast(BF16).rearrange("p (k c) -> p k c", c=C_EV)

            def k(i):
                return evb[:, i, :]

            sh = [128, C_EV]
            px = spool.tile(sh, BF16, name="px")
            py = spool.tile(sh, BF16, name="py")
            x2 = spool.tile(sh, F32, name="x2")
            d2 = spool.tile(sh, F32, name="d2")
            tt = nc.gpsimd.tensor_tensor
            tt(px[:], k(2), k(4), op=OP.mult)         # dvx * t
            tt(px[:], px[:], k(0), op=OP.add)         # + dzx
            tt(py[:], k(3), k(4), op=OP.mult)
            tt(py[:], py[:], k(1), op=OP.add)
            tt(x2[:], px[:], px[:], op=OP.mult)
            tt(d2[:], py[:], py[:], op=OP.mult)
            tt(d2[:], d2[:], x2[:], op=OP.add)
            # ACT phase order: Sqrt and Exp live in different activation
            # table sets, and every set switch costs a 1.54us table load
            # on the ACT engine. Chain all sqrts (event last), THEN all
            # exps -> exactly one mid-stream table load.
            ev_sq = nc.scalar.activation(d2[:], d2[:], AF.Sqrt,
                                         accum_out=acc[:, 12:13])
            ex_insts = []
            for m in range(M):
                ex_insts.append(nc.scalar.activation(
                    w_tiles[m][:], w_tiles[m][:], AF.Exp, scale=-1.0,
                    accum_out=acc[:, m:m + 1]))
            order = sq_insts + [ev_sq] + ex_insts
            for a2, b2 in zip(order[1:], order[:-1]):
                add_dep_helper(a2.ins, b2.ins, reason="act phase order")

            # diag-block partial sums (post-exp values live in w tiles);
            # host only needs diagA+diagB, so one [128, 256] reduce
            for m in range(M):
                nc.vector.tensor_reduce(
                    acc[:, 4 + m:5 + m], w_tiles[m][:, 0:256],
                    axis=mybir.AxisListType.X, op=OP.add,
                )

            # partition-reduce acc on PE (fp32 ones-matmul) so the out DMA
            # is one 64B descriptor instead of a 128-row DIRECT2D (~2us)
            rsum = rsq.tile([1, 16], F32, tag="rs", name="rs")
            nc.tensor.matmul(rsum[:], ones[:], acc[:], start=True, stop=True)
            osb = spool.tile([1, 16], F32, name="osb")
            nc.vector.tensor_copy(osb[:], rsum[:])
            # out doorbell on the scalar queue: its stream ends right at
            # the last accum read, while the sync stream is congested
            nc.scalar.dma_start(out_d, osb[:])
            if DEBUG_MIN:
                nc.sync.dma_start(accd_d, acc[:])

    nc.compile()
    _CACHE["nc"] = nc
    return nc


def _cheb_nodes_weights(t0, tn):
    """Discrete Gauss quadrature: M nodes/weights that reproduce
    sum_s p(t_s) over the SREF reference midpoints EXACTLY for all
    polynomials p up to degree 2M-1 (Gauss of the discrete measure).
    G(t) = sum_pairs exp(-d(t)) is analytic, so the M=2 rule already
    matches the 10-point sum to ~6e-5 relative (measured)."""
    dt = (tn - t0) / SREF
    t_s = t0 + (np.arange(SREF, dtype=np.float64) + 0.5) * dt
    mom = np.array([np.sum(t_s ** k) for k in range(2 * M + 1)])
    Amat = np.array([[mom[i + j] for j in range(M)] for i in range(M)])
    bvec = -np.array([mom[M + i] for i in range(M)])
    c = np.linalg.solve(Amat, bvec)
    coeffs = np.concatenate([c, [1.0]])
    tau = np.sort(np.roots(coeffs[::-1]).real)
    V = np.vander(tau, M, increasing=True).T
    w = np.linalg.solve(V, mom[:M])
    return tau, w


def _to_bf16(x):
    try:
        import ml_dtypes

        return x.astype(ml_dtypes.bfloat16)
    except ImportError:
        xi = x.astype(np.float32).view(np.uint32)
        r = ((xi >> 16) & 1) + 0x7FFF
        return ((xi + r) >> 16).astype(np.uint16)


def _marshal(inputs):
    z0 = np.asarray(inputs["z0"], dtype=np.float64)
    v0 = np.asarray(inputs["v0"], dtype=np.float64)
    uv = np.asarray(inputs["data_uv"], dtype=np.int64)
    tt = np.asarray(inputs["data_t"], dtype=np.float64)
    t0 = float(np.asarray(inputs["t0"]).reshape(-1)[0])
    tn = float(np.asarray(inputs["tn"]).reshape(-1)[0])

    tau, wgt = _cheb_nodes_weights(t0, tn)

    zx, zy = z0[:, 0], z0[:, 1]
    vx, vy = v0[:, 0], v0[:, 1]
    a = zx * zx + zy * zy
    b = 2.0 * (zx * vx + zy * vy)
    c = vx * vx + vy * vy
    # G_j = [1, a, b, c, zx, vx, zy, vy]
    G = np.stack([np.ones(N), a, b, c, zx, vx, zy, vy], axis=1)
    # F_i(tau) = [r, 1, t, t^2, -2x, -2tx, -2y, -2ty]
    F = np.zeros((M, N, 8))
    for m, t in enumerate(tau):
        x = zx + vx * t
        y = zy + vy * t
        r = a + b * t + c * t * t
        F[m] = np.stack(
            [r, np.ones(N), np.full(N, t), np.full(N, t * t),
             -2 * x, -2 * t * x, -2 * y, -2 * t * y], axis=1)

    E = uv.shape[0]
    assert E == NCORES * EV_PER_CORE
    zv4 = np.stack([zx, zy, vx, vy], axis=1)   # [N, 4]

    in_maps = []
    for core in range(NCORES):
        ta, tb = core, 15 - core
        # column order: [A-diag 128 | B-diag 128 | A-rest | B-rest]
        ja = np.arange(128 * ta, N)
        jb = np.arange(128 * tb, N)
        cols = np.concatenate([ja[:128], jb[:128], ja[128:], jb[128:]])
        in_a = np.concatenate([
            np.ones(128, bool), np.zeros(128, bool),
            np.ones(len(ja) - 128, bool), np.zeros(len(jb) - 128, bool)])
        assert cols.shape[0] == JC
        t2h = np.zeros((16, JC), np.float32)
        gcols = G[cols].T.astype(np.float32)            # [8, JC]
        t2h[0:8, in_a] = gcols[:, in_a]
        t2h[8:16, ~in_a] = gcols[:, ~in_a]
        l2h = np.zeros((16, M, 128), np.float32)
        for m in range(M):
            l2h[0:8, m, :] = F[m, 128 * ta:128 * ta + 128].T
            l2h[8:16, m, :] = F[m, 128 * tb:128 * tb + 128].T
        feat = np.concatenate([l2h.reshape(16, M * 128), t2h], axis=1)
        assert feat.shape == (16, FEATW)

        sl = slice(core * EV_PER_CORE, (core + 1) * EV_PER_CORE)
        us, vs, ts = uv[sl, 0], uv[sl, 1], tt[sl]
        npad = 128 * C_EV - EV_PER_CORE
        us = np.concatenate([us, np.zeros(npad, np.int64)])
        vs = np.concatenate([vs, np.zeros(npad, np.int64)])
        ts = np.concatenate([ts, np.zeros(npad)])
        # per-event differences: d(t) = ||dz + dv t||
        dz = z0[us] - z0[vs]
        dv = v0[us] - v0[vs]
        # SoA [5, 128, 196] -> [128, 5, 196]: dzx, dzy, dvx, dvy, t
        comp = np.stack(
            [dz[:, 0], dz[:, 1], dv[:, 0], dv[:, 1], ts], axis=0
        ).reshape(5, 128, C_EV).transpose(1, 0, 2)
        ev = np.ascontiguousarray(
            _to_bf16(comp.astype(np.float32))
        ).view(np.uint8).reshape(128, -1).view(np.float32)
        assert ev.shape == (128, 5 * C_EV // 2)

        in_maps.append({"feat": feat.astype(np.float32), "ev": ev})
    return in_maps, (t0, tn, E, wgt)


def _combine(core_outs, beta, t0, tn, E, wgt):
    """core_outs: list of [1, 16] float32 partial-sum tensors."""
    bsum = np.zeros(M)      # G(tau_m) totals, diag-corrected
    ev_sum = 0.0
    for o in core_outs:
        o = np.asarray(o, dtype=np.float64).reshape(-1)
        for m in range(M):
            full = o[m]
            diag = o[4 + m]
            bsum[m] += full - 0.5 * diag - 128.0
        ev_sum += o[12]
    bt = float(beta)
    dt = (tn - t0) / SREF
    non_event = np.exp(bt) * dt * float((wgt * bsum).sum())
    event_intensity = E * bt - ev_sum
    return np.float32(event_intensity - 1.0 * non_event)


def kernel(**inputs) -> np.ndarray:
    from concourse.bass_utils import run_bass_kernel_spmd

    nc = _build()
    in_maps, (t0, tn, E, wgt) = _marshal(inputs)
    res = run_bass_kernel_spmd(nc, in_maps, core_ids=list(range(NCORES)))
    beta = float(np.asarray(inputs["beta"]).reshape(-1)[0])
    out = _combine([r["outp"] for r in res.results], beta, t0, tn, E, wgt)
    return np.asarray(out, dtype=np.float32)


# revision 57
# speedup vs baseline: 1.0116x; 1.0116x over previous
"""Trainium2 Bass kernel for BasicEuclideanDistModel (gnn_message_passing).

Math:
  result = sum_e (beta - ||dz_e + dv_e t_e||)
           - dt * sum_{i<j, s} exp(beta - ||z_i(t_s) - z_j(t_s)||)

Device strategy (8 cores, one uniform SPMD program; per-core variation
lives entirely in the input DATA):

* Non-event term. The 10-point midpoint Riemann sum over t is replaced
  by a 4-node Chebyshev evaluation: G(t) = sum_pairs exp(-d(t)) is an
  analytic function of t, so sum_s G(t_s) = sum_m w_m G(tau_m) with
  Lagrange weights w (host-side; measured interp error ~5e-6 relative,
  tolerance is 2e-2). 2.5x less pairwise work than the reference grid.

  The upper triangle is cut into 16 strips (i-block b x j >= 128*b);
  core c owns strips c and 15-c, which is EXACTLY 2176 j-columns for
  every core - zero padding. d^2(i,j,t) = F_i(t).G_j as a K=16 fp32r
  inner product, where rows 0:8 carry strip-A's F/G features and rows
  8:16 strip-B's, the inactive half zeroed per column (host-packed).
  One stationary [16,128] per sample covers both strips; 5 matmuls of
  <=512 cols fill PSUM; DVE relu (PSUM f32 -> bf16), then two in-place
  ACT passes (sqrt, exp w/ fused per-partition row sums). Each strip's
  own diagonal 128-block sits at a static column (0:128, 128:256), is
  summed by a tiny DVE reduce, and the host subtracts the half-counted
  duplicates and self-pairs.

* Event term: 25000 events/core packed SoA bf16 [128, 9, 196]
  (zu, vu, zv, vv, t); pure DVE algebra -> d^2, one ACT sqrt with
  fused row-sum. Pad slots have zu=zv, t=0 -> d=0.

* Host marshalling is O(N)+O(E) data prep only (feature polynomials,
  event packing); all O(N^2 * S) and O(E) arithmetic runs on device.
  beta folded in on host: sum exp(beta-d) = e^beta sum exp(-d);
  sum(beta-d) = E beta - sum d.
"""

import os

import numpy as np


def _import_concourse():
    try:
        import concourse  # noqa: F401
    except ImportError:
        import sys

        for p in ("/opt/trn_rl_repo", "/root/.axon_site/_ro/trn_rl_repo"):
            if os.path.isdir(p) and p not in sys.path:
                sys.path.insert(0, p)


_import_concourse()

from contextlib import ExitStack  # noqa: E402

import concourse.bacc as bacc  # noqa: E402
import concourse.mybir as mybir  # noqa: E402
import concourse.tile as tile  # noqa: E402
from concourse.tile_rust import add_dep_helper  # noqa: E402

N = 2048          # nodes
NB = 16           # 128-row blocks
SREF = 10         # reference Riemann samples
M = 2             # Gauss sample nodes actually evaluated
NCORES = 8
JC = 2176         # j-columns per core: (2048-128t) + 128(t+1), exact
EV_PER_CORE = 200000 // NCORES       # 25000 real events per core
C_EV = 196        # event slots per partition (128*196 = 25088 >= 25000)
FEATW = JC + M * 128                 # combined feature input width

F32 = mybir.dt.float32
F32R = mybir.dt.float32r
BF16 = mybir.dt.bfloat16
AF = mybir.ActivationFunctionType
OP = mybir.AluOpType

# HW ACT Sqrt(x<0) = NaN (measured), and fp32r rounding pushes near-zero
# d^2 as low as -5.7e-4 (measured) - so d^2 MUST be relu'd before sqrt.
# A sqrt(x + delta) bias instead of relu costs delta/2*sum(exp(-d)/d)
# ~ 1.3e-2 relative at delta=1e-2 (measured) - too close to tolerance.
DELTA = 0.0
DEBUG_MIN = os.environ.get("BASSK_DEBUG_MIN") == "1"

_CACHE: dict = {}


def _build():
    if "nc" in _CACHE:
        return _CACHE["nc"]

    nc = bacc.Bacc(
        "TRN2", target_bir_lowering=False, debug=False, enable_asserts=False,
    )

    feat_d = nc.dram_tensor("feat", [16, FEATW], BF16, kind="ExternalInput").ap()
    ev_d = nc.dram_tensor("ev", [128, 5 * C_EV // 2], F32, kind="ExternalInput").ap()
    out_d = nc.dram_tensor("outp", [1, 16], F32, kind="ExternalOutput").ap()
    if DEBUG_MIN:
        accd_d = nc.dram_tensor("accd", [128, 16], F32,
                                kind="ExternalOutput").ap()

    with tile.TileContext(nc) as tc, ExitStack() as ctx:
        cpool = ctx.enter_context(tc.tile_pool(name="const", bufs=1))

        # feature load first (gates PE); events ride the scalar-engine
        # HWDGE queue in parallel
        # feat split across the two HWDGE queues (sync + scalar), with
        # the stationaries + first matmul columns in the sync half so
        # the first matmuls depend on one queue only; the event data
        # goes through gpsimd SWDGE with its OWN completion semaphore -
        # when it shared a queue with feat the legalized PE wait covered
        # the big event DMA and the first matmul stalled ~5us past
        # feat's landing
        feat = cpool.tile([16, FEATW], BF16)
        half = M * 128 + 1024
        fd1 = nc.sync.dma_start(feat[:, 0:half], feat_d[:, 0:half])
        fd2 = nc.scalar.dma_start(feat[:, half:FEATW], feat_d[:, half:FEATW])
        evt = cpool.tile([128, 5 * C_EV // 2], F32)
        # event data only needed mid-kernel: start it after feat so the
        # 8 cores' simultaneous feat bursts don't contend with it on HBM
        evdma = nc.gpsimd.dma_start(evt[:], ev_d)
        add_dep_helper(evdma.ins, fd1.ins, reason="ev after feat")
        add_dep_helper(evdma.ins, fd2.ins, reason="ev after feat")

        l2 = feat[:, 0:M * 128].rearrange(
            "p (m c) -> p m c", c=128
        )                                                # [16, M, 128] F rows
        t2 = feat[:, M * 128:FEATW]                      # [16, 2176] G cols

        acc = cpool.tile([128, 16], F32)
        nc.vector.memset(acc[:], 0.0)
        ones = cpool.tile([128, 1], F32)
        nc.vector.memset(ones[:], 1.0)

        # ---------------- pairwise: matmul -> relu -> sqrt -> exp ----
        with tc.tile_pool(name="bigq", bufs=3, space="PSUM") as bigq, \
                tc.tile_pool(name="smallq", bufs=1, space="PSUM") as smallq, \
                tc.tile_pool(name="rsq", bufs=1, space="PSUM") as rsq, \
                tc.tile_pool(name="wp", bufs=M) as wpool, \
                tc.tile_pool(name="sp", bufs=1) as spool:
            w_tiles = []
            sq_insts = []
            for m in range(M):
                lm = l2[:, m, :]
                w = wpool.tile([128, JC], BF16, tag="w", name="w")
                qa = bigq.tile([128, 2, 512], F32, tag="q", name="qa")
                nc.tensor.matmul(qa[:, 0, :], lm, t2[:, 0:512],
                                 start=True, stop=True)
                nc.tensor.matmul(qa[:, 1, :], lm, t2[:, 512:1024],
                                 start=True, stop=True)
                qb = bigq.tile([128, 2, 512], F32, tag="q", name="qb")
                nc.tensor.matmul(qb[:, 0, :], lm, t2[:, 1024:1536],
                                 start=True, stop=True)
                nc.tensor.matmul(qb[:, 1, :], lm, t2[:, 1536:2048],
                                 start=True, stop=True)
                qs = smallq.tile([128, 128], F32, tag="qs", name="qs")
                nc.tensor.matmul(qs[:], lm, t2[:, 2048:JC],
                                 start=True, stop=True)
                # relu clamps fp32r-rounding negatives ahead of ACT sqrt.
                # Sample 0 front is fine-chunked so ACT starts ~1us
                # earlier; the qb+qs sqrt merges into one [1152] op
                # (per-op overhead is ~300ns on ACT).
                if m == 0:
                    nc.vector.tensor_scalar_max(
                        w[:, 0:512], qa[:, 0, :], 0.0)
                    sq_insts.append(nc.scalar.activation(
                        w[:, 0:512], w[:, 0:512], AF.Sqrt))
                    nc.vector.tensor_scalar_max(
                        w[:, 512:1024], qa[:, 1, :], 0.0)
                    sq_insts.append(nc.scalar.activation(
                        w[:, 512:1024], w[:, 512:1024], AF.Sqrt))
                else:
                    nc.vector.tensor_scalar_max(w[:, 0:1024], qa[:], 0.0)
                    sq_insts.append(nc.scalar.activation(
                        w[:, 0:1024], w[:, 0:1024], AF.Sqrt))
                nc.vector.tensor_scalar_max(w[:, 1024:2048], qb[:], 0.0)
                nc.vector.tensor_scalar_max(w[:, 2048:JC], qs[:], 0.0)
                sq_insts.append(nc.scalar.activation(
                    w[:, 1024:JC], w[:, 1024:JC], AF.Sqrt))
                if DEBUG_MIN:
                    nc.vector.tensor_reduce(
                        acc[:, 13 + m:14 + m],
                        qa[:].rearrange("p a b -> p (a b)"),
                        axis=mybir.AxisListType.X, op=OP.min)
                w_tiles.append(w)

            # -------- event term (Pool-engine algebra + one ACT sqrt) ----
            # host packs per-event [dzx, dzy, dvx, dvy, t]; the algebra
            # runs on gpsimd (SBUF-only ops) so it's ready as soon as the
            # event DMA lands, in parallel with the DVE relus. d^2 comes
            # from SQUARES, so it is non-negative by construction (ACT
            # Sqrt of a negative is NaN).
            evb = evt[:].bitcast(BF16).rearrange("p (k c) -> p k c", c=C_EV)

            def k(i):
                return evb[:, i, :]

            sh = [128, C_EV]
            px = spool.tile(sh, BF16, name="px")
            py = spool.tile(sh, BF16, name="py")
            x2 = spool.tile(sh, F32, name="x2")
            d2 = spool.tile(sh, F32, name="d2")
            tt = nc.gpsimd.tensor_tensor
            tt(px[:], k(2), k(4), op=OP.mult)         # dvx * t
            tt(px[:], px[:], k(0), op=OP.add)         # + dzx
            tt(py[:], k(3), k(4), op=OP.mult)
            tt(py[:], py[:], k(1), op=OP.add)
            tt(x2[:], px[:], px[:], op=OP.mult)
            tt(d2[:], py[:], py[:], op=OP.mult)
            tt(d2[:], d2[:], x2[:], op=OP.add)
            # ACT phase order: Sqrt and Exp live in different activation
            # table sets, and every set switch costs a 1.54us table load
            # on the ACT engine. Chain all sqrts (event last), THEN all
            # exps -> exactly one mid-stream table load.
            ev_sq = nc.scalar.activation(d2[:], d2[:], AF.Sqrt,
                                         accum_out=acc[:, 12:13])
            ex_insts = []
            for m in range(M):
                ex_insts.append(nc.scalar.activation(
                    w_tiles[m][:], w_tiles[m][:], AF.Exp, scale=-1.0,
                    accum_out=acc[:, m:m + 1]))
            order = sq_insts + [ev_sq] + ex_insts
            for a2, b2 in zip(order[1:], order[:-1]):
                add_dep_helper(a2.ins, b2.ins, reason="act phase order")

            # diag-block partial sums (post-exp values live in w tiles);
            # host only needs diagA+diagB, so one [128, 256] reduce
            for m in range(M):
                nc.vector.tensor_reduce(
                    acc[:, 4 + m:5 + m], w_tiles[m][:, 0:256],
                    axis=mybir.AxisListType.X, op=OP.add,
                )

            # partition-reduce acc on PE (fp32 ones-matmul) so the out DMA
            # is one 64B descriptor instead of a 128-row DIRECT2D (~2us)
            rsum = rsq.tile([1, 16], F32, tag="rs", name="rs")
            nc.tensor.matmul(rsum[:], ones[:], acc[:], start=True, stop=True)
            osb = spool.tile([1, 16], F32, name="osb")
            nc.vector.tensor_copy(osb[:], rsum[:])
            # out doorbell on the scalar queue: its stream ends right at
            # the last accum read, while the sync stream is congested
            nc.scalar.dma_start(out_d, osb[:])
            if DEBUG_MIN:
                nc.sync.dma_start(accd_d, acc[:])

    nc.compile()
    _CACHE["nc"] = nc
    return nc


def _cheb_nodes_weights(t0, tn):
    """Discrete Gauss quadrature: M nodes/weights that reproduce
    sum_s p(t_s) over the SREF reference midpoints EXACTLY for all
    polynomials p up to degree 2M-1 (Gauss of the discrete measure).
    G(t) = sum_pairs exp(-d(t)) is analytic, so the M=2 rule already
    matches the 10-point sum to ~6e-5 relative (measured)."""
    dt = (tn - t0) / SREF
    t_s = t0 + (np.arange(SREF, dtype=np.float64) + 0.5) * dt
    mom = np.array([np.sum(t_s ** k) for k in range(2 * M + 1)])
    Amat = np.array([[mom[i + j] for j in range(M)] for i in range(M)])
    bvec = -np.array([mom[M + i] for i in range(M)])
    c = np.linalg.solve(Amat, bvec)
    coeffs = np.concatenate([c, [1.0]])
    tau = np.sort(np.roots(coeffs[::-1]).real)
    V = np.vander(tau, M, increasing=True).T
    w = np.linalg.solve(V, mom[:M])
    return tau, w


def _to_bf16(x):
    try:
        import ml_dtypes

        return x.astype(ml_dtypes.bfloat16)
    except ImportError:
        xi = x.astype(np.float32).view(np.uint32)
        r = ((xi >> 16) & 1) + 0x7FFF
        return ((xi + r) >> 16).astype(np.uint16)


def _marshal(inputs):
    z0 = np.asarray(inputs["z0"], dtype=np.float64)
    v0 = np.asarray(inputs["v0"], dtype=np.float64)
    uv = np.asarray(inputs["data_uv"], dtype=np.int64)
    tt = np.asarray(inputs["data_t"], dtype=np.float64)
    t0 = float(np.asarray(inputs["t0"]).reshape(-1)[0])
    tn = float(np.asarray(inputs["tn"]).reshape(-1)[0])

    tau, wgt = _cheb_nodes_weights(t0, tn)

    zx, zy = z0[:, 0], z0[:, 1]
    vx, vy = v0[:, 0], v0[:, 1]
    a = zx * zx + zy * zy
    b = 2.0 * (zx * vx + zy * vy)
    c = vx * vx + vy * vy
    # G_j = [1, a, b, c, zx, vx, zy, vy]
    G = np.stack([np.ones(N), a, b, c, zx, vx, zy, vy], axis=1)
    # F_i(tau) = [r, 1, t, t^2, -2x, -2tx, -2y, -2ty]
    F = np.zeros((M, N, 8))
    for m, t in enumerate(tau):
        x = zx + vx * t
        y = zy + vy * t
        r = a + b * t + c * t * t
        F[m] = np.stack(
            [r, np.ones(N), np.full(N, t), np.full(N, t * t),
             -2 * x, -2 * t * x, -2 * y, -2 * t * y], axis=1)

    E = uv.shape[0]
    assert E == NCORES * EV_PER_CORE
    zv4 = np.stack([zx, zy, vx, vy], axis=1)   # [N, 4]

    in_maps = []
    for core in range(NCORES):
        ta, tb = core, 15 - core
        # column order: [A-diag 128 | B-diag 128 | A-rest | B-rest]
        ja = np.arange(128 * ta, N)
        jb = np.arange(128 * tb, N)
        cols = np.concatenate([ja[:128], jb[:128], ja[128:], jb[128:]])
        in_a = np.concatenate([
            np.ones(128, bool), np.zeros(128, bool),
            np.ones(len(ja) - 128, bool), np.zeros(len(jb) - 128, bool)])
        assert cols.shape[0] == JC
        t2h = np.zeros((16, JC), np.float32)
        gcols = G[cols].T.astype(np.float32)            # [8, JC]
        t2h[0:8, in_a] = gcols[:, in_a]
        t2h[8:16, ~in_a] = gcols[:, ~in_a]
        l2h = np.zeros((16, M, 128), np.float32)
        for m in range(M):
            l2h[0:8, m, :] = F[m, 128 * ta:128 * ta + 128].T
            l2h[8:16, m, :] = F[m, 128 * tb:128 * tb + 128].T
        feat = _to_bf16(
            np.concatenate([l2h.reshape(16, M * 128), t2h], axis=1))
        assert feat.shape == (16, FEATW)

        sl = slice(core * EV_PER_CORE, (core + 1) * EV_PER_CORE)
        us, vs, ts = uv[sl, 0], uv[sl, 1], tt[sl]
        npad = 128 * C_EV - EV_PER_CORE
        us = np.concatenate([us, np.zeros(npad, np.int64)])
        vs = np.concatenate([vs, np.zeros(npad, np.int64)])
        ts = np.concatenate([ts, np.zeros(npad)])
        # per-event differences: d(t) = ||dz + dv t||
        dz = z0[us] - z0[vs]
        dv = v0[us] - v0[vs]
        # SoA [5, 128, 196] -> [128, 5, 196]: dzx, dzy, dvx, dvy, t
        comp = np.stack(
            [dz[:, 0], dz[:, 1], dv[:, 0], dv[:, 1], ts], axis=0
        ).reshape(5, 128, C_EV).transpose(1, 0, 2)
        ev = np.ascontiguousarray(
            _to_bf16(comp.astype(np.float32))
        ).view(np.uint8).reshape(128, -1).view(np.float32)
        assert ev.shape == (128, 5 * C_EV // 2)

        in_maps.append({"feat": feat, "ev": ev})
    return in_maps, (t0, tn, E, wgt)


def _combine(core_outs, beta, t0, tn, E, wgt):
    """core_outs: list of [1, 16] float32 partial-sum tensors."""
    bsum = np.zeros(M)      # G(tau_m) totals, diag-corrected
    ev_sum = 0.0
    for o in core_outs:
        o = np.asarray(o, dtype=np.float64).reshape(-1)
        for m in range(M):
            full = o[m]
            diag = o[4 + m]
            bsum[m] += full - 0.5 * diag - 128.0
        ev_sum += o[12]
    bt = float(beta)
    dt = (tn - t0) / SREF
    non_event = np.exp(bt) * dt * float((wgt * bsum).sum())
    event_intensity = E * bt - ev_sum
    return np.float32(event_intensity - 1.0 * non_event)


def kernel(**inputs) -> np.ndarray:
    from concourse.bass_utils import run_bass_kernel_spmd

    nc = _build()
    in_maps, (t0, tn, E, wgt) = _marshal(inputs)
    res = run_bass_kernel_spmd(nc, in_maps, core_ids=list(range(NCORES)))
    beta = float(np.asarray(inputs["beta"]).reshape(-1)[0])
    out = _combine([r["outp"] for r in res.results], beta, t0, tn, E, wgt)
    return np.asarray(out, dtype=np.float32)


# revision 58
# speedup vs baseline: 1.0353x; 1.0235x over previous
"""Trainium2 Bass kernel for BasicEuclideanDistModel (gnn_message_passing).

Math:
  result = sum_e (beta - ||dz_e + dv_e t_e||)
           - dt * sum_{i<j, s} exp(beta - ||z_i(t_s) - z_j(t_s)||)

Device strategy (8 cores, one uniform SPMD program; per-core variation
lives entirely in the input DATA):

* Non-event term. The 10-point midpoint Riemann sum over t is replaced
  by a 4-node Chebyshev evaluation: G(t) = sum_pairs exp(-d(t)) is an
  analytic function of t, so sum_s G(t_s) = sum_m w_m G(tau_m) with
  Lagrange weights w (host-side; measured interp error ~5e-6 relative,
  tolerance is 2e-2). 2.5x less pairwise work than the reference grid.

  The upper triangle is cut into 16 strips (i-block b x j >= 128*b);
  core c owns strips c and 15-c, which is EXACTLY 2176 j-columns for
  every core - zero padding. d^2(i,j,t) = F_i(t).G_j as a K=16 fp32r
  inner product, where rows 0:8 carry strip-A's F/G features and rows
  8:16 strip-B's, the inactive half zeroed per column (host-packed).
  One stationary [16,128] per sample covers both strips; 5 matmuls of
  <=512 cols fill PSUM; DVE relu (PSUM f32 -> bf16), then two in-place
  ACT passes (sqrt, exp w/ fused per-partition row sums). Each strip's
  own diagonal 128-block sits at a static column (0:128, 128:256), is
  summed by a tiny DVE reduce, and the host subtracts the half-counted
  duplicates and self-pairs.

* Event term: 25000 events/core packed SoA bf16 [128, 9, 196]
  (zu, vu, zv, vv, t); pure DVE algebra -> d^2, one ACT sqrt with
  fused row-sum. Pad slots have zu=zv, t=0 -> d=0.

* Host marshalling is O(N)+O(E) data prep only (feature polynomials,
  event packing); all O(N^2 * S) and O(E) arithmetic runs on device.
  beta folded in on host: sum exp(beta-d) = e^beta sum exp(-d);
  sum(beta-d) = E beta - sum d.
"""

import os

import numpy as np


def _import_concourse():
    try:
        import concourse  # noqa: F401
    except ImportError:
        import sys

        for p in ("/opt/trn_rl_repo", "/root/.axon_site/_ro/trn_rl_repo"):
            if os.path.isdir(p) and p not in sys.path:
                sys.path.insert(0, p)


_import_concourse()

from contextlib import ExitStack  # noqa: E402

import concourse.bacc as bacc  # noqa: E402
import concourse.mybir as mybir  # noqa: E402
import concourse.tile as tile  # noqa: E402
from concourse.tile_rust import add_dep_helper  # noqa: E402

N = 2048          # nodes
NB = 16           # 128-row blocks
SREF = 10         # reference Riemann samples
M = 2             # Gauss sample nodes actually evaluated
NCORES = 8
JC = 2176         # j-columns per core: (2048-128t) + 128(t+1), exact
EV_PER_CORE = 200000 // NCORES       # 25000 real events per core
C_EV = 196        # event slots per partition (128*196 = 25088 >= 25000)
FEATW = JC + M * 128                 # combined feature input width

F32 = mybir.dt.float32
F32R = mybir.dt.float32r
BF16 = mybir.dt.bfloat16
AF = mybir.ActivationFunctionType
OP = mybir.AluOpType

# HW ACT Sqrt(x<0) = NaN (measured), and fp32r rounding pushes near-zero
# d^2 as low as -5.7e-4 (measured) - so d^2 MUST be relu'd before sqrt.
# A sqrt(x + delta) bias instead of relu costs delta/2*sum(exp(-d)/d)
# ~ 1.3e-2 relative at delta=1e-2 (measured) - too close to tolerance.
DELTA = 0.0
DEBUG_MIN = os.environ.get("BASSK_DEBUG_MIN") == "1"

_CACHE: dict = {}


def _build():
    if "nc" in _CACHE:
        return _CACHE["nc"]

    nc = bacc.Bacc(
        "TRN2", target_bir_lowering=False, debug=False, enable_asserts=False,
    )

    feat_d = nc.dram_tensor("feat", [16, FEATW], BF16, kind="ExternalInput").ap()
    ev_d = nc.dram_tensor("ev", [128, 5 * C_EV // 2], F32, kind="ExternalInput").ap()
    out_d = nc.dram_tensor("outp", [1, 16], F32, kind="ExternalOutput").ap()
    if DEBUG_MIN:
        accd_d = nc.dram_tensor("accd", [128, 16], F32,
                                kind="ExternalOutput").ap()

    with tile.TileContext(nc) as tc, ExitStack() as ctx:
        cpool = ctx.enter_context(tc.tile_pool(name="const", bufs=1))

        # feature load first (gates PE); events ride the scalar-engine
        # HWDGE queue in parallel
        # feat split across the two HWDGE queues (sync + scalar), with
        # the stationaries + first matmul columns in the sync half so
        # the first matmuls depend on one queue only; the event data
        # goes through gpsimd SWDGE with its OWN completion semaphore -
        # when it shared a queue with feat the legalized PE wait covered
        # the big event DMA and the first matmul stalled ~5us past
        # feat's landing
        feat = cpool.tile([16, FEATW], BF16)
        half = M * 128 + 1024
        fd1 = nc.sync.dma_start(feat[:, 0:half], feat_d[:, 0:half])
        fd2 = nc.scalar.dma_start(feat[:, half:FEATW], feat_d[:, half:FEATW])
        evt = cpool.tile([128, 5 * C_EV // 2], F32)
        # event data only needed mid-kernel: start it after the first
        # feat half so the 8 cores' simultaneous feat bursts don't
        # contend with it on HBM (full serialization pushed the landing
        # into the ACT stream's ev_sq slot on the unluckiest core)
        evdma = nc.gpsimd.dma_start(evt[:], ev_d)
        add_dep_helper(evdma.ins, fd1.ins, reason="ev after feat half")
        del fd2

        l2 = feat[:, 0:M * 128].rearrange(
            "p (m c) -> p m c", c=128
        )                                                # [16, M, 128] F rows
        t2 = feat[:, M * 128:FEATW]                      # [16, 2176] G cols

        acc = cpool.tile([128, 16], F32)
        nc.vector.memset(acc[:], 0.0)
        ones = cpool.tile([128, 1], F32)
        nc.vector.memset(ones[:], 1.0)

        # ---------------- pairwise: matmul -> relu -> sqrt -> exp ----
        with tc.tile_pool(name="bigq", bufs=3, space="PSUM") as bigq, \
                tc.tile_pool(name="smallq", bufs=1, space="PSUM") as smallq, \
                tc.tile_pool(name="rsq", bufs=1, space="PSUM") as rsq, \
                tc.tile_pool(name="wp", bufs=M) as wpool, \
                tc.tile_pool(name="sp", bufs=1) as spool:
            w_tiles = []
            sq_insts = []
            for m in range(M):
                lm = l2[:, m, :]
                w = wpool.tile([128, JC], BF16, tag="w", name="w")
                qa = bigq.tile([128, 2, 512], F32, tag="q", name="qa")
                nc.tensor.matmul(qa[:, 0, :], lm, t2[:, 0:512],
                                 start=True, stop=True)
                nc.tensor.matmul(qa[:, 1, :], lm, t2[:, 512:1024],
                                 start=True, stop=True)
                qb = bigq.tile([128, 2, 512], F32, tag="q", name="qb")
                nc.tensor.matmul(qb[:, 0, :], lm, t2[:, 1024:1536],
                                 start=True, stop=True)
                nc.tensor.matmul(qb[:, 1, :], lm, t2[:, 1536:2048],
                                 start=True, stop=True)
                qs = smallq.tile([128, 128], F32, tag="qs", name="qs")
                nc.tensor.matmul(qs[:], lm, t2[:, 2048:JC],
                                 start=True, stop=True)
                # relu clamps fp32r-rounding negatives ahead of ACT sqrt.
                # Sample 0 front is fine-chunked so ACT starts ~1us
                # earlier; the qb+qs sqrt merges into one [1152] op
                # (per-op overhead is ~300ns on ACT).
                if m == 0:
                    nc.vector.tensor_scalar_max(
                        w[:, 0:512], qa[:, 0, :], 0.0)
                    sq_insts.append(nc.scalar.activation(
                        w[:, 0:512], w[:, 0:512], AF.Sqrt))
                    nc.vector.tensor_scalar_max(
                        w[:, 512:1024], qa[:, 1, :], 0.0)
                    sq_insts.append(nc.scalar.activation(
                        w[:, 512:1024], w[:, 512:1024], AF.Sqrt))
                else:
                    nc.vector.tensor_scalar_max(w[:, 0:1024], qa[:], 0.0)
                    sq_insts.append(nc.scalar.activation(
                        w[:, 0:1024], w[:, 0:1024], AF.Sqrt))
                nc.vector.tensor_scalar_max(w[:, 1024:2048], qb[:], 0.0)
                nc.vector.tensor_scalar_max(w[:, 2048:JC], qs[:], 0.0)
                sq_insts.append(nc.scalar.activation(
                    w[:, 1024:JC], w[:, 1024:JC], AF.Sqrt))
                if DEBUG_MIN:
                    nc.vector.tensor_reduce(
                        acc[:, 13 + m:14 + m],
                        qa[:].rearrange("p a b -> p (a b)"),
                        axis=mybir.AxisListType.X, op=OP.min)
                w_tiles.append(w)

            # -------- event term (Pool-engine algebra + one ACT sqrt) ----
            # host packs per-event [dzx, dzy, dvx, dvy, t]; the algebra
            # runs on gpsimd (SBUF-only ops) so it's ready as soon as the
            # event DMA lands, in parallel with the DVE relus. d^2 comes
            # from SQUARES, so it is non-negative by construction (ACT
            # Sqrt of a negative is NaN).
            evb = evt[:].bitcast(BF16).rearrange("p (k c) -> p k c", c=C_EV)

            def k(i):
                return evb[:, i, :]

            sh = [128, C_EV]
            px = spool.tile(sh, BF16, name="px")
            py = spool.tile(sh, BF16, name="py")
            x2 = spool.tile(sh, F32, name="x2")
            d2 = spool.tile(sh, F32, name="d2")
            tt = nc.gpsimd.tensor_tensor
            tt(px[:], k(2), k(4), op=OP.mult)         # dvx * t
            tt(px[:], px[:], k(0), op=OP.add)         # + dzx
            tt(py[:], k(3), k(4), op=OP.mult)
            tt(py[:], py[:], k(1), op=OP.add)
            tt(x2[:], px[:], px[:], op=OP.mult)
            tt(d2[:], py[:], py[:], op=OP.mult)
            tt(d2[:], d2[:], x2[:], op=OP.add)
            # ACT phase order: Sqrt and Exp live in different activation
            # table sets, and every set switch costs a 1.54us table load
            # on the ACT engine. Chain all sqrts (event last), THEN all
            # exps -> exactly one mid-stream table load.
            ev_sq = nc.scalar.activation(d2[:], d2[:], AF.Sqrt,
                                         accum_out=acc[:, 12:13])
            ex_insts = []
            for m in range(M):
                ex_insts.append(nc.scalar.activation(
                    w_tiles[m][:], w_tiles[m][:], AF.Exp, scale=-1.0,
                    accum_out=acc[:, m:m + 1]))
            order = sq_insts + [ev_sq] + ex_insts
            for a2, b2 in zip(order[1:], order[:-1]):
                add_dep_helper(a2.ins, b2.ins, reason="act phase order")

            # diag-block partial sums (post-exp values live in w tiles);
            # host only needs diagA+diagB, so one [128, 256] reduce
            for m in range(M):
                nc.vector.tensor_reduce(
                    acc[:, 4 + m:5 + m], w_tiles[m][:, 0:256],
                    axis=mybir.AxisListType.X, op=OP.add,
                )

            # partition-reduce acc on PE (fp32 ones-matmul) so the out DMA
            # is one 64B descriptor instead of a 128-row DIRECT2D (~2us)
            rsum = rsq.tile([1, 16], F32, tag="rs", name="rs")
            nc.tensor.matmul(rsum[:], ones[:], acc[:], start=True, stop=True)
            osb = spool.tile([1, 16], F32, name="osb")
            nc.vector.tensor_copy(osb[:], rsum[:])
            # out doorbell on the scalar queue: its stream ends right at
            # the last accum read, while the sync stream is congested
            nc.scalar.dma_start(out_d, osb[:])
            if DEBUG_MIN:
                nc.sync.dma_start(accd_d, acc[:])

    nc.compile()
    _CACHE["nc"] = nc
    return nc


def _cheb_nodes_weights(t0, tn):
    """Discrete Gauss quadrature: M nodes/weights that reproduce
    sum_s p(t_s) over the SREF reference midpoints EXACTLY for all
    polynomials p up to degree 2M-1 (Gauss of the discrete measure).
    G(t) = sum_pairs exp(-d(t)) is analytic, so the M=2 rule already
    matches the 10-point sum to ~6e-5 relative (measured)."""
    dt = (tn - t0) / SREF
    t_s = t0 + (np.arange(SREF, dtype=np.float64) + 0.5) * dt
    mom = np.array([np.sum(t_s ** k) for k in range(2 * M + 1)])
    Amat = np.array([[mom[i + j] for j in range(M)] for i in range(M)])
    bvec = -np.array([mom[M + i] for i in range(M)])
    c = np.linalg.solve(Amat, bvec)
    coeffs = np.concatenate([c, [1.0]])
    tau = np.sort(np.roots(coeffs[::-1]).real)
    V = np.vander(tau, M, increasing=True).T
    w = np.linalg.solve(V, mom[:M])
    return tau, w


def _to_bf16(x):
    try:
        import ml_dtypes

        return x.astype(ml_dtypes.bfloat16)
    except ImportError:
        xi = x.astype(np.float32).view(np.uint32)
        r = ((xi >> 16) & 1) + 0x7FFF
        return ((xi + r) >> 16).astype(np.uint16)


def _marshal(inputs):
    z0 = np.asarray(inputs["z0"], dtype=np.float64)
    v0 = np.asarray(inputs["v0"], dtype=np.float64)
    uv = np.asarray(inputs["data_uv"], dtype=np.int64)
    tt = np.asarray(inputs["data_t"], dtype=np.float64)
    t0 = float(np.asarray(inputs["t0"]).reshape(-1)[0])
    tn = float(np.asarray(inputs["tn"]).reshape(-1)[0])

    tau, wgt = _cheb_nodes_weights(t0, tn)

    zx, zy = z0[:, 0], z0[:, 1]
    vx, vy = v0[:, 0], v0[:, 1]
    a = zx * zx + zy * zy
    b = 2.0 * (zx * vx + zy * vy)
    c = vx * vx + vy * vy
    # G_j = [1, a, b, c, zx, vx, zy, vy]
    G = np.stack([np.ones(N), a, b, c, zx, vx, zy, vy], axis=1)
    # F_i(tau) = [r, 1, t, t^2, -2x, -2tx, -2y, -2ty]
    F = np.zeros((M, N, 8))
    for m, t in enumerate(tau):
        x = zx + vx * t
        y = zy + vy * t
        r = a + b * t + c * t * t
        F[m] = np.stack(
            [r, np.ones(N), np.full(N, t), np.full(N, t * t),
             -2 * x, -2 * t * x, -2 * y, -2 * t * y], axis=1)

    E = uv.shape[0]
    assert E == NCORES * EV_PER_CORE
    zv4 = np.stack([zx, zy, vx, vy], axis=1)   # [N, 4]

    in_maps = []
    for core in range(NCORES):
        ta, tb = core, 15 - core
        # column order: [A-diag 128 | B-diag 128 | A-rest | B-rest]
        ja = np.arange(128 * ta, N)
        jb = np.arange(128 * tb, N)
        cols = np.concatenate([ja[:128], jb[:128], ja[128:], jb[128:]])
        in_a = np.concatenate([
            np.ones(128, bool), np.zeros(128, bool),
            np.ones(len(ja) - 128, bool), np.zeros(len(jb) - 128, bool)])
        assert cols.shape[0] == JC
        t2h = np.zeros((16, JC), np.float32)
        gcols = G[cols].T.astype(np.float32)            # [8, JC]
        t2h[0:8, in_a] = gcols[:, in_a]
        t2h[8:16, ~in_a] = gcols[:, ~in_a]
        l2h = np.zeros((16, M, 128), np.float32)
        for m in range(M):
            l2h[0:8, m, :] = F[m, 128 * ta:128 * ta + 128].T
            l2h[8:16, m, :] = F[m, 128 * tb:128 * tb + 128].T
        feat = _to_bf16(
            np.concatenate([l2h.reshape(16, M * 128), t2h], axis=1))
        assert feat.shape == (16, FEATW)

        sl = slice(core * EV_PER_CORE, (core + 1) * EV_PER_CORE)
        us, vs, ts = uv[sl, 0], uv[sl, 1], tt[sl]
        npad = 128 * C_EV - EV_PER_CORE
        us = np.concatenate([us, np.zeros(npad, np.int64)])
        vs = np.concatenate([vs, np.zeros(npad, np.int64)])
        ts = np.concatenate([ts, np.zeros(npad)])
        # per-event differences: d(t) = ||dz + dv t||
        dz = z0[us] - z0[vs]
        dv = v0[us] - v0[vs]
        # SoA [5, 128, 196] -> [128, 5, 196]: dzx, dzy, dvx, dvy, t
        comp = np.stack(
            [dz[:, 0], dz[:, 1], dv[:, 0], dv[:, 1], ts], axis=0
        ).reshape(5, 128, C_EV).transpose(1, 0, 2)
        ev = np.ascontiguousarray(
            _to_bf16(comp.astype(np.float32))
        ).view(np.uint8).reshape(128, -1).view(np.float32)
        assert ev.shape == (128, 5 * C_EV // 2)

        in_maps.append({"feat": feat, "ev": ev})
    return in_maps, (t0, tn, E, wgt)


def _combine(core_outs, beta, t0, tn, E, wgt):
    """core_outs: list of [1, 16] float32 partial-sum tensors."""
    bsum = np.zeros(M)      # G(tau_m) totals, diag-corrected
    ev_sum = 0.0
    for o in core_outs:
        o = np.asarray(o, dtype=np.float64).reshape(-1)
        for m in range(M):
            full = o[m]
            diag = o[4 + m]
            bsum[m] += full - 0.5 * diag - 128.0
        ev_sum += o[12]
    bt = float(beta)
    dt = (tn - t0) / SREF
    non_event = np.exp(bt) * dt * float((wgt * bsum).sum())
    event_intensity = E * bt - ev_sum
    return np.float32(event_intensity - 1.0 * non_event)


def kernel(**inputs) -> np.ndarray:
    from concourse.bass_utils import run_bass_kernel_spmd

    nc = _build()
    in_maps, (t0, tn, E, wgt) = _marshal(inputs)
    res = run_bass_kernel_spmd(nc, in_maps, core_ids=list(range(NCORES)))
    beta = float(np.asarray(inputs["beta"]).reshape(-1)[0])
    out = _combine([r["outp"] for r in res.results], beta, t0, tn, E, wgt)
    return np.asarray(out, dtype=np.float32)
